# revision 23
# baseline (speedup 1.0000x reference)
"""MoE (8 experts, top-2) TRN2 kernel — expert-parallel with token routing
and two-slot load balancing.

The host computes top-2 routing indices in fp32 (data movement only — all
arithmetic that produces output values runs on device) and distributes the
32768 (token, expert) pairs across 8 cores in 128-token blocks:
  slot 1: the first K1 blocks of the core's own expert,
  slot 2: up to K2 blocks of some overloaded expert's remainder,
so every core processes the same capacity B = K1+K2 blocks (vs max expert
load if unbalanced). Both slots' expert weights are per-core inputs; segment
boundaries are compile-time constants, so one program serves all cores.

Each core: bf16 gating logits for its C=128*B tokens -> softmax (exp on
scalar engine, PE transpose to token-major, vector reduce), own-slot
expert's raw prob = combine weight -> 2-layer gelu FFN in bf16 -> rows
scaled by the combine weight -> [C, H] fp32 partial. Host scatter-adds the
two expert partials per token.
"""

import sys
import types

sys.path.insert(0, "/opt/trn_rl_repo")

import numpy as np
import ml_dtypes

try:
    import antenv.axon_hooks  # noqa: F401
except ImportError:
    try:
        import antenv
        import trn_agent_boot.trn_boot as _tb

        _hook = _tb._ntff_profile_via_ctypes("/opt/axon/libaxon_pjrt.so")
        _m = types.ModuleType("antenv.axon_hooks")
        _m.get_axon_ntff_profile_hook = lambda: _hook
        _m.set_axon_ntff_profile_hook = lambda h: None
        sys.modules["antenv.axon_hooks"] = _m
        antenv.axon_hooks = _m
    except Exception:
        pass

import concourse.bacc as bacc
import concourse.mybir as mybir
from concourse import bass, bass_utils
from concourse.tile import TileContext
from concourse.masks import make_identity

E = 8
H = 512
F = 2048
T = 8 * 2048
BF16 = mybir.dt.bfloat16
F32 = mybir.dt.float32

_CACHE = {}
LAST_RESULT = None


def _group_segs(g, gl, K1):
    """Maximal same-slot runs of 128-blocks inside group g ([col0, ncols, slot])."""
    segs = []
    for k in range(gl // 128):
        sl = 0 if 4 * g + k < K1 else 1
        if segs and segs[-1][2] == sl:
            segs[-1][1] += 128
        else:
            segs.append([k * 128, 128, sl])
    return segs


def _build(B, K1, K2):
    """Compile the per-core kernel: capacity B blocks, slot boundary at K1."""
    C = B * 128
    G = (C + 511) // 512
    nc = bacc.Bacc(debug=False)

    xbt = nc.declare_dram_parameter("xbt", [128, 4, C], BF16, isOutput=False)
    wg = nc.declare_dram_parameter("wg", [128, 4, E], BF16, isOutput=False)
    bgq = nc.declare_dram_parameter("bgq", [E, 1], F32, isOutput=False)
    w1a = nc.declare_dram_parameter("w1a", [128, 4, F], BF16, isOutput=False)
    b1a = nc.declare_dram_parameter("b1a", [128, F // 128], F32, isOutput=False)
    w2a = nc.declare_dram_parameter("w2a", [128, F // 128, H], BF16, isOutput=False)
    b2a = nc.declare_dram_parameter("b2a", [128, H], F32, isOutput=False)
    if K2 > 0:
        w1b = nc.declare_dram_parameter("w1b", [128, 4, F], BF16, isOutput=False)
        b1b = nc.declare_dram_parameter("b1b", [128, F // 128], F32, isOutput=False)
        w2b = nc.declare_dram_parameter(
            "w2b", [128, F // 128, H], BF16, isOutput=False
        )
        b2b = nc.declare_dram_parameter("b2b", [128, H], F32, isOutput=False)
    ypart = nc.declare_dram_parameter("ypart", [C, H], F32, isOutput=True)

    def glen(g):
        return min(512, C - 512 * g)

    with TileContext(nc) as tc:
        with (
            tc.tile_pool(name="const", bufs=1) as constp,
            tc.tile_pool(name="work", bufs=3) as work,
            tc.tile_pool(name="psA", bufs=3, space="PSUM") as psA,
            tc.tile_pool(name="psB", bufs=3, space="PSUM") as psB,
        ):
            gate = work

            # token chunk 0 first so gating + mm1(0) start early, then the
            # slot-A weights, then remaining chunks, then slot-B weights
            x_sb = constp.tile([128, 4, C], BF16)
            nc.sync.dma_start(out=x_sb[:, :, 0:512], in_=xbt[:, :, 0:512])
            wg_sb = constp.tile([128, 4, E], BF16)
            nc.sync.dma_start(out=wg_sb[:], in_=wg[:])
            # w1a split into column chunks: mm1(0)'s first chains only need
            # the first chunk, so the PE starts earlier
            w1_sb = [constp.tile([128, 4, F], BF16, name="w1a_sb")]
            for fq in range(4):
                nc.sync.dma_start(
                    out=w1_sb[0][:, :, fq * 512 : (fq + 1) * 512],
                    in_=w1a[:, :, fq * 512 : (fq + 1) * 512],
                )
            bgq_sb = constp.tile([E, 1], F32)
            nc.sync.dma_start(out=bgq_sb[:], in_=bgq[:])
            b1_sb = [constp.tile([128, F // 128], F32, name="b1a_sb")]
            nc.sync.dma_start(out=b1_sb[0][:], in_=b1a[:])
            ident = constp.tile([128, 128], F32)
            make_identity(nc, ident[:])
            for g in range(1, G):
                nc.sync.dma_start(
                    out=x_sb[:, :, 512 * g : 512 * g + glen(g)],
                    in_=xbt[:, :, 512 * g : 512 * g + glen(g)],
                )
            w2_sb = [constp.tile([128, F // 128, H], BF16, name="w2a_sb")]
            nc.sync.dma_start(out=w2_sb[0][:], in_=w2a[:])
            b2_sb = [constp.tile([128, H], F32, name="b2a_sb")]
            nc.sync.dma_start(out=b2_sb[0][:], in_=b2a[:])
            if K2 > 0:
                w1_sb.append(constp.tile([128, 4, F], BF16, name="w1b_sb"))
                nc.sync.dma_start(out=w1_sb[1][:], in_=w1b[:])
                b1_sb.append(constp.tile([128, F // 128], F32, name="b1b_sb"))
                nc.sync.dma_start(out=b1_sb[1][:], in_=b1b[:])
                w2_sb.append(constp.tile([128, F // 128, H], BF16, name="w2b_sb"))
                nc.sync.dma_start(out=w2_sb[1][:], in_=w2b[:])
                b2_sb.append(constp.tile([128, H], F32, name="b2b_sb"))
                nc.sync.dma_start(out=b2_sb[1][:], in_=b2b[:])

            comb_all = constp.tile([128, 4, G], F32)  # [tok%128, tokblk%4, group]

            ex_tiles = {}

            def emit_gate_mms(g):
                gl = glen(g)
                lp = psB.tile([E, 512], F32, tag="lp", bufs=1)
                for hc in range(4):
                    nc.tensor.matmul(
                        lp[:, :gl],
                        wg_sb[:, hc, :],
                        x_sb[:, hc, 512 * g : 512 * g + gl],
                        start=(hc == 0),
                        stop=(hc == 3),
                    )
                ex = gate.tile([E, 512], F32, tag="ex")
                nc.scalar.activation(
                    ex[:, :gl],
                    lp[:, :gl],
                    mybir.ActivationFunctionType.Exp,
                    bias=bgq_sb[:],
                    scale=1.0,
                )
                ex_tiles[g] = ex

            def emit_gate_fin(g):
                gl = glen(g)
                nk = gl // 128
                ex = ex_tiles.pop(g)
                tp = psA.tile([128, 4, E], F32, tag="tp", bufs=1)
                for k in range(nk):
                    nc.tensor.transpose(
                        tp[:, k, :],
                        ex[:, k * 128 : (k + 1) * 128],
                        ident[:E, :E],
                    )
                den = gate.tile([128, 4], F32, tag="den")
                nc.vector.tensor_reduce(
                    den[:, :nk],
                    tp[:, :nk, :],
                    axis=mybir.AxisListType.X,
                    op=mybir.AluOpType.add,
                )
                rcp = gate.tile([128, 4], F32, tag="rcp")
                nc.vector.reciprocal(rcp[:, :nk], den[:, :nk])
                for c0, ncols, sl in _group_segs(g, gl, K1):
                    k0, k1 = c0 // 128, (c0 + ncols) // 128
                    nc.vector.tensor_tensor(
                        out=comb_all[:, k0:k1, g],
                        in0=tp[:, k0:k1, sl],
                        in1=rcp[:, k0:k1],
                        op=mybir.AluOpType.mult,
                    )

            def emit_mm1(g):
                gl = glen(g)
                hb = work.tile([128, F // 128, 512], BF16, tag="hb")
                for c0, ncols, sl in _group_segs(g, gl, K1):
                    for ft in range(F // 128):
                        hp = psA.tile([128, 512], F32, tag="mmA")
                        for hc in range(4):
                            nc.tensor.matmul(
                                hp[:, :ncols],
                                w1_sb[sl][:, hc, ft * 128 : (ft + 1) * 128],
                                x_sb[:, hc, 512 * g + c0 : 512 * g + c0 + ncols],
                                start=(hc == 0),
                                stop=(hc == 3),
                            )
                        nc.scalar.activation(
                            hb[:, ft, c0 : c0 + ncols],
                            hp[:, :ncols],
                            mybir.ActivationFunctionType.Gelu_apprx_tanh,
                            bias=b1_sb[sl][:, ft : ft + 1],
                            scale=1.0,
                        )
                return hb

            def emit_mm2(g, hb):
                gl = glen(g)
                for st in range(gl // 128):
                    sl = 0 if 4 * g + st < K1 else 1
                    yp = psB.tile([128, 512], F32, tag="mmB")
                    for fc in range(F // 128):
                        nc.tensor.matmul(
                            yp[:],
                            hb[:, fc, st * 128 : (st + 1) * 128],
                            w2_sb[sl][:, fc, :],
                            start=(fc == 0),
                            stop=(fc == F // 128 - 1),
                        )
                    yt = work.tile([128, H], F32, tag="yt")
                    nc.vector.tensor_tensor(
                        out=yt[:], in0=yp[:], in1=b2_sb[sl][:], op=mybir.AluOpType.add
                    )
                    ys = work.tile([128, H], F32, tag="ys")
                    nc.vector.tensor_scalar_mul(
                        ys[:], yt[:], comb_all[:, st, g : g + 1]
                    )
                    nc.sync.dma_start(
                        out=ypart[512 * g + st * 128 : 512 * g + (st + 1) * 128, :],
                        in_=ys[:],
                    )

            # prologue: gating group 0, then mm1(0) keeps the PE busy while
            # the remaining token chunks + weights stream in. Gating is
            # software-pipelined: group g's matmuls overlap group g-1's
            # transposes so the PE never waits on the exp activation.
            emit_gate_mms(0)
            hbs = [emit_mm1(0)]
            for g in range(1, G):
                emit_gate_mms(g)
                emit_gate_fin(g - 1)
            emit_gate_fin(G - 1)
            if G > 1:
                hbs.append(emit_mm1(1))
            for g in range(G):
                if g + 2 < G:
                    hbs.append(emit_mm1(g + 2))
                emit_mm2(g, hbs[g])
    nc.compile()
    return nc


def _route(x, Wg, bg):
    """fp32 top-2 routing indices (host-side, drives the gather only)."""
    xf = np.ascontiguousarray(np.asarray(x, dtype=np.float32).reshape(T, H))
    logits = xf @ np.asarray(Wg, np.float32) + np.asarray(bg, np.float32)
    m = logits.max(-1, keepdims=True)
    p = np.exp(logits - m)
    p /= p.sum(-1, keepdims=True)
    topi = np.argsort(-p, axis=-1, kind="stable")[:, :2]
    idxs = [np.nonzero((topi == e).any(axis=1))[0] for e in range(E)]
    return xf, idxs


def _plan(idxs):
    """Choose capacity/slots and the block assignment for each core.

    Returns (B, K1, K2, per-core list of (slot2_expert, slot2_token_indices)).
    """
    blocks = [(len(i) + 127) // 128 for i in idxs]
    Btot = sum(blocks)
    B = (Btot + E - 1) // E
    K1 = min(blocks)
    K2 = B - K1
    chunks = []  # (expert, start_block, nblocks)
    if K2 > 0:
        for e in range(E):
            r = blocks[e] - K1
            s = K1
            while r > 0:
                take = min(r, K2)
                chunks.append((e, s, take))
                s += take
                r -= take
    if len(chunks) > E:
        # fallback: classic expert-per-core, capacity = max expert load
        B, K1, K2, chunks = max(blocks), max(blocks), 0, []

    # assign chunks to cores, never a chunk of the core's own expert
    assign = [None] * E
    free = list(range(E))
    for ch in sorted(chunks, key=lambda c: c[0]):
        pick = next((c for c in free if c != ch[0]), None)
        if pick is None:  # only own core left: swap with an earlier one
            pick = free[0]
            for j in range(E):
                if assign[j] is not None and assign[j][0] != pick and j != ch[0]:
                    assign[pick] = assign[j]
                    assign[j] = None
                    pick = j
                    break
        free.remove(pick)
        assign[pick] = ch
    return B, K1, K2, assign


def _prep_inputs(xf, idxs, plan, Wg, bg, W1, b1, W2, b2):
    B, K1, K2, assign = plan
    C = B * 128
    Wg = np.asarray(Wg, dtype=np.float32)
    bg = np.asarray(bg, dtype=np.float32)
    W1 = np.asarray(W1, dtype=np.float32)
    b1 = np.asarray(b1, dtype=np.float32)
    W2 = np.asarray(W2, dtype=np.float32)
    b2 = np.asarray(b2, dtype=np.float32)

    def wpack(e):
        return (
            np.ascontiguousarray(
                np.transpose(W1[e].reshape(4, 128, F), (1, 0, 2)).astype(
                    ml_dtypes.bfloat16
                )
            ),
            np.ascontiguousarray(b1[e].reshape(F // 128, 128).T),
            np.ascontiguousarray(
                np.transpose(W2[e].reshape(F // 128, 128, H), (1, 0, 2)).astype(
                    ml_dtypes.bfloat16
                )
            ),
            np.ascontiguousarray(np.broadcast_to(b2[e][None, :], (128, H)).copy()),
        )

    in_maps = []
    row_maps = []
    for e in range(E):
        rows = np.full(C, -1, np.int64)
        n1 = min(len(idxs[e]), 128 * K1)
        rows[:n1] = idxs[e][:n1]
        eb = e
        if assign[e] is not None:
            eb, sblk, take = assign[e]
            tok = idxs[eb][128 * sblk : min(len(idxs[eb]), 128 * (sblk + take))]
            rows[128 * K1 : 128 * K1 + len(tok)] = tok
        row_maps.append(rows)

        perm = [e] + ([eb] if eb != e else []) + [
            j for j in range(E) if j != e and j != eb
        ]
        wg_t = np.ascontiguousarray(
            np.transpose(Wg[:, perm].reshape(4, 128, E), (1, 0, 2)).astype(
                ml_dtypes.bfloat16
            )
        )
        bgq = np.ascontiguousarray(bg[perm].reshape(E, 1))
        xq = np.zeros((C, H), np.float32)
        valid = rows >= 0
        xq[valid] = xf[rows[valid]]
        xbt = np.ascontiguousarray(
            np.transpose(xq.T.reshape(4, 128, C), (1, 0, 2)).astype(ml_dtypes.bfloat16)
        )
        w1e, b1e, w2e, b2e = wpack(e)
        m = {
            "xbt": xbt,
            "wg": wg_t,
            "bgq": bgq,
            "w1a": w1e,
            "b1a": b1e,
            "w2a": w2e,
            "b2a": b2e,
        }
        if K2 > 0:
            w1x, b1x, w2x, b2x = wpack(eb) if eb != e else (w1e, b1e, w2e, b2e)
            m.update({"w1b": w1x, "b1b": b1x, "w2b": w2x, "b2b": b2x})
        in_maps.append(m)
    return in_maps, row_maps


def kernel(x, Wg, bg, W1, b1, W2, b2):
    global LAST_RESULT
    xf, idxs = _route(x, Wg, bg)
    plan = _plan(idxs)
    B, K1, K2, _ = plan

    key = (B, K1, K2)
    if key not in _CACHE:
        _CACHE[key] = _build(B, K1, K2)
    nc = _CACHE[key]
    in_maps, row_maps = _prep_inputs(xf, idxs, plan, Wg, bg, W1, b1, W2, b2)
    import os

    trace = bool(os.environ.get("BASS_TRACE"))
    res = bass_utils.run_bass_kernel_spmd(
        nc, in_maps, core_ids=list(range(E)), trace=trace
    )
    LAST_RESULT = res
    out = np.zeros((T, H), np.float32)
    n1 = 128 * K1
    for e in range(E):
        rows = row_maps[e]
        yp = res.results[e]["ypart"]
        # scatter each slot separately: token indices are unique within a
        # slot, but a token routed to both of this core's experts would be
        # dropped by a single buffered fancy-index +=
        for sl in (slice(0, n1), slice(n1, None)):
            r = rows[sl]
            valid = r >= 0
            if valid.any():
                out[r[valid]] += yp[sl][valid]
    return out.reshape(8, 2048, H)
